# revision 17
# baseline (speedup 1.0000x reference)
"""Distributed Trainium2 (Bass/Tile) kernel for a Qwen3-style attention layer.

Full layer: QKV proj -> per-head RMSNorm (q,k) -> RoPE -> GQA SDPA -> o_proj.

Sharding over 8 NeuronCores:
  - tensor-parallel across heads for QKV+attention: core c owns q-heads
    [4c, 4c+4) and kv-head c; hidden_states replicated.
  - AllToAll exchanges attention context so each core ends with all 4096
    context dims for a 256-token slice; o_proj is then token-parallel with
    Wo replicated (streamed). Output: per-core [256, 4096] chunks that the
    host concatenates. No all-reduce needed.

Pipeline (single in-order emission per engine; Tile handles cross-engine
sync):
  phase A: proj0 (k, v, q0..q3 chains; fine-grained first-weight DMA)
  phase B: proj1 chains interleaved with attn0 heads and attn1 heads
           h0..h2 (softmax exp on Act paces each head, so PE fills with
           proj chains); a2a0 fires after attn0.h3
  phase C: attn1.h3 -> fire a2a1; o_proj group-streamed (Wo ring buffer,
           batch-0 groups first so the collective overlaps them)

Perf notes (axon CoreSim): matmul pitch is ~259ns while DMA streams
(HBM/SBUF contention), ~216ns when quiet -- so the wins are continuous PE
occupancy + never letting a dependent DMA head-of-line-block the queue.
RoPE's half-swap runs on a DMA of the drained psum copy (rs is
partition-uniform, so swap(q*w*rs) == swap(q)*swap(w)*rs).
"""

import numpy as np
import ml_dtypes

import concourse.bass as bass
import concourse.mybir as mybir
from concourse import bacc
from concourse.tile import TileContext
from concourse.bass_utils import run_bass_kernel_spmd

F32 = mybir.dt.float32
BF16 = mybir.dt.bfloat16
BF16_NP = ml_dtypes.bfloat16

N_CORES = 8

FULL_CFG = dict(B=2, S=1024, HID=4096, H=32, KV=8, D=128, eps=1e-6)


def build_program(B=2, S=1024, HID=4096, H=32, KV=8, D=128, eps=1e-6):
    cores = N_CORES
    assert D == 128 and H % cores == 0 and KV == cores and B == 2
    HQ = H // cores            # q heads per core
    HH = HQ // 2               # heads per a2a half
    T = B * S                  # total tokens
    HCH = HID // 128           # hidden-dim chunks of 128
    TT = min(512, S)           # projection token tile (within batch)
    TPB = S // TT              # projection tiles per batch
    KB = S // 128              # key blocks per batch
    QT = min(512, S)           # attention q tile
    QTB = S // QT              # q tiles per batch
    TC = T // cores            # output tokens per core
    TCB = TC // B              # per-batch token slice per core
    ICH = (H * D) // 128       # o_proj contraction chunks (32)
    OH = min(512, HID // 2)    # o_proj hid tile width
    NHG = HID // OH            # number of hid groups (8)
    WOB = 8                    # wo ring: quarter-group tiles
    scale = float(D) ** -0.5
    MULT = mybir.AluOpType.mult
    SW = QTB * QT              # full q row per batch (== S)

    nc = bacc.Bacc("TRN2", target_bir_lowering=False, debug=False,
                   num_devices=cores)

    hT = nc.dram_tensor("hT", [B, HCH // 4, 128, 4 * S], BF16,
                        kind="ExternalInput")
    wq = nc.dram_tensor("wq", [HQ, 128, HCH * 128], BF16, kind="ExternalInput")
    wk = nc.dram_tensor("wk", [128, HCH * 128], BF16, kind="ExternalInput")
    wv = nc.dram_tensor("wv", [128, HCH * 128], BF16, kind="ExternalInput")
    wo = nc.dram_tensor("wo", [NHG, 128, ICH * OH], BF16,
                        kind="ExternalInput")
    cosT = nc.dram_tensor("cosT", [128, S], BF16, kind="ExternalInput")
    csinT = nc.dram_tensor("csinT", [128, S], BF16, kind="ExternalInput")
    nrm = nc.dram_tensor("nrm", [128, 4], F32, kind="ExternalInput")
    out = nc.dram_tensor("out", [TC, HID], F32, kind="ExternalOutput")

    with TileContext(nc) as tc:
        with (
            tc.tile_pool(name="const", bufs=1) as cp,
            tc.tile_pool(name="dram", bufs=1, space="DRAM") as dramp,
            tc.tile_pool(name="qkv", bufs=1) as p_qkv,
            tc.tile_pool(name="work", bufs=2) as p_work,
            tc.tile_pool(name="pt", bufs=2) as p_pt,
            tc.tile_pool(name="psum", bufs=1, space="PSUM") as ps_all,
        ):
            ones_s = cp.tile([128, 128], BF16)
            nc.vector.memset(ones_s[:, :], 1.0)
            eps_s = cp.tile([128, 1], F32)
            nc.vector.memset(eps_s[:, :], eps)
            cos_s = cp.tile([128, S], BF16)
            nc.scalar.dma_start(out=cos_s[:, :], in_=cosT[:, :])
            csin_s = cp.tile([128, S], BF16)
            nc.scalar.dma_start(out=csin_s[:, :], in_=csinT[:, :])
            nrm_t = cp.tile([128, 4], F32, tag="nrm")
            nc.scalar.dma_start(out=nrm_t[:, :], in_=nrm[:, :])
            nrm_w = {nm: nrm_t[:, i:i + 1]
                     for i, nm in enumerate(("qw", "kw", "qwsw", "kwsw"))}

            a2a_in = [[dramp.tile([cores * HH * 128, TCB], BF16,
                                  tag=f"a2ai{b}_{p}", name=f"a2ai{b}_{p}")
                       for p in range(2)] for b in range(B)]
            a2a_out = [[dramp.tile([cores * HH * 128, TCB], BF16,
                                   tag=f"a2ao{b}_{p}", name=f"a2ao{b}_{p}")
                        for p in range(2)] for b in range(B)]

            qT_s = p_qkv.tile([128, HQ * T], BF16, tag="qT")
            kT_s = p_qkv.tile([128, T], BF16, tag="kT")
            vnat_s = p_qkv.tile([128, T], BF16, tag="vnat")
            ctxT_s = p_qkv.tile([128, HQ * T], BF16, tag="ctxT")

            # ---------------- projection building blocks ----------------
            def proj_chains(b, ob, w_t, hch):
                """One output block (128 rows): TPB chains + epilogues.

                ob: 0..HQ-1 = q heads, HQ = k, HQ+1 = v.
                """
                for tt in range(TPB):
                    ps = ps_all.tile([128, TT], F32, tag="mm", name="ps",
                                     bufs=2)
                    for ch in range(HCH):
                        nc.tensor.matmul(
                            ps[:, :],
                            lhsT=w_t[:, ch * 128:(ch + 1) * 128],
                            rhs=hch[ch // 4][:, (ch % 4) * S +
                                             tt * TT:(ch % 4) * S +
                                             (tt + 1) * TT],
                            start=(ch == 0), stop=(ch == HCH - 1))
                    tg = b * S + tt * TT
                    pos = tt * TT
                    if ob <= HQ:
                        is_q = ob < HQ
                        dst = (qT_s[:, ob * T + tg: ob * T + tg + TT]
                               if is_q else kT_s[:, tg: tg + TT])
                        wcol = nrm_w["qw" if is_q else "kw"]
                        wcsw = nrm_w["qwsw" if is_q else "kwsw"]
                        # drain psum once; half-swap the drained copy
                        # immediately (rs is partition-uniform so the
                        # norm+rope math commutes with the swap)
                        ot = p_work.tile([128, TT], F32, tag="ot")
                        nc.scalar.copy(ot[:, :], ps[:, :])
                        otw = p_work.tile([128, TT], F32, tag="otw")
                        nc.scalar.dma_start(out=otw[0:64, :],
                                            in_=ot[64:128, :])
                        nc.scalar.dma_start(out=otw[64:128, :],
                                            in_=ot[0:64, :])
                        sq = p_work.tile([128, TT], BF16, tag="sq")
                        nc.vector.tensor_mul(sq[:, :], ot[:, :], ot[:, :])
                        ssq = ps_all.tile([128, TT], F32, tag="ctx",
                                          name="ssq", bufs=2)
                        nc.tensor.matmul(ssq[:, :], lhsT=ones_s[:, :],
                                         rhs=sq[:, :], start=True, stop=True)
                        std = p_work.tile([128, TT], F32, tag="std")
                        nc.scalar.activation(
                            std[:, :], ssq[:, :],
                            mybir.ActivationFunctionType.Sqrt,
                            bias=eps_s[:, :], scale=1.0 / D)
                        rs = p_work.tile([128, TT], F32, tag="rs")
                        nc.vector.reciprocal_approx_fast(rs[:, :], std[:, :])
                        qn = p_work.tile([128, TT], BF16, tag="qn")
                        nc.vector.scalar_tensor_tensor(
                            qn[:, :], in0=ot[:, :], scalar=wcol[:, :],
                            in1=rs[:, :], op0=MULT, op1=MULT)
                        qsw = p_work.tile([128, TT], BF16, tag="qsw")
                        nc.vector.scalar_tensor_tensor(
                            qsw[:, :], in0=otw[:, :], scalar=wcsw[:, :],
                            in1=rs[:, :], op0=MULT, op1=MULT)
                        t1 = p_work.tile([128, TT], BF16, tag="t1")
                        nc.vector.tensor_mul(t1[:, :], qn[:, :],
                                             cos_s[:, pos: pos + TT])
                        t2 = p_work.tile([128, TT], BF16, tag="t2")
                        nc.vector.tensor_mul(t2[:, :], qsw[:, :],
                                             csin_s[:, pos: pos + TT])
                        nc.vector.tensor_add(dst, t1[:, :], t2[:, :])
                    else:
                        vt = p_work.tile([128, TT], BF16, tag="vt")
                        nc.scalar.copy(vt[:, :], ps[:, :])
                        for tb in range(TT // 128):
                            tbg = tg // 128 + tb
                            nc.scalar.dma_start_transpose(
                                vnat_s[:, tbg * 128:(tbg + 1) * 128],
                                vt[:, tb * 128:(tb + 1) * 128])

            def load_h(b, p_hid, split_first=False):
                hch = []
                for c4 in range(HCH // 4):
                    t_ = p_hid.tile([128, 4 * S], BF16, tag="hid",
                                    name="hid")
                    if c4 == 0 and split_first:
                        for c in range(4):
                            nc.sync.dma_start(
                                out=t_[:, c * S:(c + 1) * S],
                                in_=hT[b, 0, :, c * S:(c + 1) * S])
                    else:
                        nc.sync.dma_start(out=t_[:, :], in_=hT[b, c4])
                    hch.append(t_)
                return hch

            def load_w(ob, p_w, split=1):
                w_t = p_w.tile([128, HCH * 128], BF16, tag="w", name="w")
                srcw = (wq[ob] if ob < HQ else
                        (wk[:, :] if ob == HQ else wv[:, :]))
                step = (HCH * 128) // split
                for s0 in range(0, HCH * 128, step):
                    nc.sync.dma_start(out=w_t[:, s0:s0 + step],
                                      in_=srcw[:, s0:s0 + step])
                return w_t

            # ---------------- attention building blocks -----------------
            def attn_head(b, h):
                """Scores+exp, ctx, softmax-divide, ship for one head."""
                qoff = h * T + b * S
                pt_t = p_pt.tile([128, KB * S], BF16, tag="pT", name="pT")
                for kb in range(KB):
                    sps = ps_all.tile([128, SW], F32, tag="sps",
                                      name="sps", bufs=2)
                    for qt in range(QTB):
                        nc.tensor.matmul(
                            sps[:, qt * QT:(qt + 1) * QT],
                            lhsT=kT_s[:, b * S + kb * 128:
                                      b * S + (kb + 1) * 128],
                            rhs=qT_s[:, qoff + qt * QT: qoff + (qt + 1) * QT],
                            start=True, stop=True)
                    nc.scalar.activation(
                        pt_t[:, kb * SW:(kb + 1) * SW], sps[:, :],
                        mybir.ActivationFunctionType.Exp, scale=scale)
                ctxs = [ps_all.tile([128, QT], F32, tag="ctx", name="ctx",
                                    bufs=2) for _ in range(QTB)]
                for kb in range(KB):
                    tbg = (b * S) // 128 + kb
                    for qt in range(QTB):
                        nc.tensor.matmul(
                            ctxs[qt][:, :],
                            lhsT=vnat_s[:, tbg * 128:(tbg + 1) * 128],
                            rhs=pt_t[:, kb * SW + qt * QT:
                                     kb * SW + (qt + 1) * QT],
                            start=(kb == 0), stop=(kb == KB - 1))
                for qt in range(QTB):
                    po = qt * QT
                    dd = []
                    for g in range(4):
                        d_ = p_work.tile([128, QT], BF16, tag=f"d{g}",
                                         bufs=1)
                        nc.vector.tensor_add(
                            d_[:, :], pt_t[:, po + 2 * g * SW:
                                           po + 2 * g * SW + QT],
                            pt_t[:, po + (2 * g + 1) * SW:
                                 po + (2 * g + 1) * SW + QT])
                        dd.append(d_)
                    e0 = p_work.tile([128, QT], BF16, tag="e0", bufs=1)
                    nc.vector.tensor_add(e0[:, :], dd[0][:, :], dd[1][:, :])
                    e1 = p_work.tile([128, QT], BF16, tag="e1", bufs=1)
                    nc.vector.tensor_add(e1[:, :], dd[2][:, :], dd[3][:, :])
                    denp = p_work.tile([128, QT], BF16, tag="denp", bufs=1)
                    nc.vector.tensor_add(denp[:, :], e0[:, :], e1[:, :])
                    dps = ps_all.tile([128, QT], F32, tag="mm", name="dps",
                                      bufs=2)
                    nc.tensor.matmul(dps[:, :], lhsT=ones_s[:, :],
                                     rhs=denp[:, :], start=True, stop=True)
                    rec = p_work.tile([128, QT], F32, tag="rec")
                    nc.vector.reciprocal_approx_fast(rec[:, :], dps[:, :])
                    nc.vector.tensor_mul(
                        ctxT_s[:, qoff + qt * QT: qoff + (qt + 1) * QT],
                        ctxs[qt][:, :], rec[:, :])
                # ship this head's context into its half's a2a buffer
                pi, hh = h // HH, h % HH
                dstv = a2a_in[b][pi][:, :].rearrange(
                    "(j hh p) t -> hh p j t", hh=HH, p=128)
                srcv = ctxT_s[:, qoff: qoff + S].rearrange(
                    "p (j t) -> p j t", t=TCB)
                nc.sync.dma_start(out=dstv[hh], in_=srcv)

            def fire_a2a(b, pi):
                nc.gpsimd.collective_compute(
                    "AllToAll", mybir.AluOpType.bypass,
                    replica_groups=[list(range(cores))],
                    ins=[a2a_in[b][pi].opt()],
                    outs=[a2a_out[b][pi].opt()])

            # ---- phase A+B: proj0, attn0, proj1 (hid/w pools open) ------
            with (
                tc.tile_pool(name="hid", bufs=HCH // 4) as p_hid,
                tc.tile_pool(name="wts", bufs=2) as p_w,
            ):
                OBS = [HQ, HQ + 1, 0, 1, 2, 3]   # k, v, q0..q3

                def proj(b, split_first):
                    w_next = load_w(OBS[0], p_w, split=(4 if split_first
                                                        else 1))
                    hch = load_h(b, p_hid, split_first=split_first)
                    for i, ob in enumerate(OBS):
                        w_t = w_next
                        if i + 1 < len(OBS):
                            w_next = load_w(OBS[i + 1], p_w)
                        proj_chains(b, ob, w_t, hch)

                sc_ = nc.enter_named_scope("proj0", True)[0]
                proj(0, True)
                nc.leave_named_scope("proj0", sc_, True)

                sc_ = nc.enter_named_scope("attn0", True)[0]
                attn_head(0, 0)
                attn_head(0, 1)
                fire_a2a(0, 0)
                attn_head(0, 2)
                attn_head(0, 3)
                fire_a2a(0, 1)
                nc.leave_named_scope("attn0", sc_, True)

                sc_ = nc.enter_named_scope("proj1", True)[0]
                proj(1, False)
                nc.leave_named_scope("proj1", sc_, True)

            # ---- phase C: attn1 (wo prefetch under it) + o_proj ---------
            with (
                tc.tile_pool(name="wo", bufs=1) as p_wo,
                tc.tile_pool(name="cx", bufs=1) as p_cx,
                tc.tile_pool(name="oo", bufs=4) as p_oo,
            ):
                cx_s = [[p_cx.tile([128, cores * HH * TCB], BF16,
                                   tag=f"cx{b}_{p}", name=f"cx{b}_{p}")
                         for p in range(2)] for b in range(B)]

                def load_cx(b, eng=None):
                    eng = eng or nc.sync
                    for pi in range(2):
                        eng.dma_start(
                            out=cx_s[b][pi][:, :].rearrange(
                                "p (blk t) -> p blk t", t=TCB),
                            in_=a2a_out[b][pi][:, :].rearrange(
                                "(blk p) t -> p blk t", p=128))

                def cx_lhsT(b, ic):
                    j, h = ic // HQ, ic % HQ
                    pi, blk = h // HH, (ic // HQ) * HH + h % HH
                    return cx_s[b][pi][:, blk * TCB:(blk + 1) * TCB]

                def load_wo_g(g, eng=None):
                    eng = eng or nc.sync
                    QW = (ICH // 4) * OH
                    wts = []
                    for q in range(4):
                        wo_t = p_wo.tile([128, QW], BF16, tag="wo",
                                         name="wo", bufs=WOB)
                        eng.dma_start(out=wo_t[:, :],
                                      in_=wo[g, :, q * QW:(q + 1) * QW])
                        wts.append(wo_t)
                    return wts

                def oproj_unit(g, b, wts):
                    pso = ps_all.tile([TCB, OH], F32, tag="mm", name="pso",
                                      bufs=2)
                    ICQ = ICH // 4
                    for ic in range(ICH):
                        nc.tensor.matmul(
                            pso[:, :],
                            lhsT=cx_lhsT(b, ic),
                            rhs=wts[ic // ICQ][:, (ic % ICQ) * OH:
                                               (ic % ICQ + 1) * OH],
                            start=(ic == 0), stop=(ic == ICH - 1))
                    ot = p_oo.tile([TCB, OH], F32, tag="oout", name="oout")
                    nc.vector.tensor_copy(ot[:, :], pso[:, :])
                    nc.sync.dma_start(
                        out=out[b * TCB:(b + 1) * TCB,
                                g * OH:(g + 1) * OH],
                        in_=ot[:, :])

                sc_ = nc.enter_named_scope("attn1", True)[0]
                load_cx(0)                 # a2a0 landed during proj1
                attn_head(1, 0)
                attn_head(1, 1)
                fire_a2a(1, 0)
                wts_g = {0: load_wo_g(0)}
                attn_head(1, 2)
                wts_g[1] = load_wo_g(1, eng=nc.scalar)
                attn_head(1, 3)
                fire_a2a(1, 1)
                nc.leave_named_scope("attn1", sc_, True)

                sc_ = nc.enter_named_scope("oproj", True)[0]
                oproj_unit(0, 0, wts_g[0])
                oproj_unit(1, 0, wts_g[1])
                load_cx(1, eng=nc.scalar)
                oproj_unit(0, 1, wts_g[0])
                wts_g[2] = load_wo_g(2, eng=nc.scalar)
                oproj_unit(1, 1, wts_g[1])
                wts_g[3] = load_wo_g(3, eng=nc.scalar)
                for g in range(2, NHG):
                    oproj_unit(g, 0, wts_g[g])
                    oproj_unit(g, 1, wts_g[g])
                    if g + 2 < NHG:
                        wts_g[g + 2] = load_wo_g(g + 2, eng=nc.scalar)
                nc.leave_named_scope("oproj", sc_, True)

    nc.compile()
    return nc


def host_prep(inputs, B=2, S=1024, HID=4096, H=32, KV=8, D=128, eps=1e-6):
    """Shard + lay out the full inputs into per-core in_maps."""
    cores = N_CORES
    HQ = H // cores
    HCH = HID // 128
    ICH = (H * D) // 128

    hs = np.ascontiguousarray(inputs["hidden_states"], dtype=np.float32)
    fc = np.asarray(inputs["freqs_cis"], dtype=np.float32)
    Wq = np.asarray(inputs["Wq"], dtype=np.float32)
    Wk = np.asarray(inputs["Wk"], dtype=np.float32)
    Wv = np.asarray(inputs["Wv"], dtype=np.float32)
    Wo = np.asarray(inputs["Wo"], dtype=np.float32)
    qnw = np.asarray(inputs["q_norm_w"], dtype=np.float32)
    knw = np.asarray(inputs["k_norm_w"], dtype=np.float32)

    # hidden^T 4-chunk tiles: hT[b, c4, p, c*S+s] = hs[b, s, (4*c4+c)*128+p]
    S = hs.shape[1]
    h4 = hs.transpose(0, 2, 1).reshape(B, HCH // 4, 4, 128, S)
    h4 = h4.transpose(0, 1, 3, 2, 4)
    hT = np.ascontiguousarray(
        h4.reshape(B, HCH // 4, 128, 4 * S)).astype(BF16_NP)

    cos, sin, nsin = fc[0], fc[1], fc[2]      # [S, D]
    cosT = np.ascontiguousarray(cos.T).astype(BF16_NP)    # [128, S]
    csinT = np.concatenate([nsin.T[0:64], sin.T[64:128]], axis=0)
    csinT = np.ascontiguousarray(csinT).astype(BF16_NP)

    def col(v):
        return np.ascontiguousarray(v.reshape(128, 1).astype(np.float32))

    nrm_pack = np.stack([
        qnw, knw,
        np.concatenate([qnw[64:], qnw[:64]]),
        np.concatenate([knw[64:], knw[:64]]),
    ], axis=1).astype(np.float32)
    nrm_pack = np.ascontiguousarray(nrm_pack)

    # Wo^T per hid-group: wog[g, p, ic*OH+j] = Wo[g*OH+j, ic*128+p]
    OH = 512
    NHG = HID // OH
    w4 = Wo.T.reshape(ICH, 128, NHG, OH).transpose(2, 1, 0, 3)
    woT = np.ascontiguousarray(w4.reshape(NHG, 128, ICH * OH)).astype(BF16_NP)

    def prep_w(Wm, nblocks):
        # [nblocks, p, ch*128] with w[ob, p, ch*128+j] = Wm[ob*128+j, ch*128+p]
        a = Wm.reshape(nblocks, 128, HCH, 128).transpose(0, 3, 2, 1)
        return np.ascontiguousarray(a.reshape(nblocks, 128, HCH * 128)) \
            .astype(BF16_NP)

    in_maps = []
    for c in range(cores):
        Wq_c = Wq[c * HQ * D:(c + 1) * HQ * D]
        Wk_c = Wk[c * D:(c + 1) * D]
        Wv_c = Wv[c * D:(c + 1) * D]
        in_maps.append({
            "hT": hT,
            "wq": prep_w(Wq_c, HQ),
            "wk": prep_w(Wk_c, 1)[0],
            "wv": prep_w(Wv_c, 1)[0],
            "wo": woT,
            "cosT": cosT,
            "csinT": csinT,
            "nrm": nrm_pack,
        })
    return in_maps


def gather_output(results, B=2, S=1024, HID=4096, **_):
    cores = N_CORES
    TCB = (B * S) // cores // B
    out = np.empty((B, S, HID), dtype=np.float32)
    for c in range(cores):
        o = results[c]["out"]
        for b in range(B):
            out[b, c * TCB:(c + 1) * TCB] = o[b * TCB:(b + 1) * TCB]
    return out


_NC_CACHE = {}


def kernel(**inputs) -> np.ndarray:
    cfg = FULL_CFG
    key = tuple(sorted(cfg.items()))
    if key not in _NC_CACHE:
        _NC_CACHE[key] = build_program(**cfg)
    nc = _NC_CACHE[key]
    in_maps = host_prep(inputs, **cfg)
    res = run_bass_kernel_spmd(nc, in_maps, core_ids=list(range(N_CORES)))
    return gather_output(res.results, **cfg)
